# revision 40
# baseline (speedup 1.0000x reference)
"""GPT-2 transformer block on 8 Trainium2 NeuronCores.

Sharding: core c = (batch b = c//2, rank r = c%2).  Pairs (2b, 2b+1) share a
batch: each core computes ln1 + qkv for its 6 of 12 heads over the full
sequence (T=2048), causal attention in transposed layout, an intra-pair
AllGather of per-head outputs per 512-token quarter, then token-parallel
aproj + ln2 + FFN where rank r owns quarters {r, r+2}.  The FFN work for the
first owned quarter is interleaved into the scalar-bound (exp) attention
stream of quarters 2-3 so the PE array never idles.  Dense matmuls (qkv,
aproj, fc, mproj) run in fp8-e4m3 DoubleRow (pairs of adjacent 128-chunks
along the contraction dim); attention scores/AV stay bf16.  Weights are
pre-scaled by 64 before the fp8 cast (avoids the subnormal range) and the
1/64 is folded into the consuming activation/vector op; the attention score
scale 1/sqrt(64) is folded into the exp activation's scale.  LayerNorm
gains/biases and b_aproj are folded into weights / the residual input on the
host.
"""

import numpy as np
import ml_dtypes

import concourse.bass as bass
import concourse.tile as tile
from concourse import mybir
from concourse.alu_op_type import AluOpType
from concourse.masks import make_identity
from concourse.bass_utils import run_bass_kernel_spmd

BF16 = mybir.dt.bfloat16
F32 = mybir.dt.float32
F8 = mybir.dt.float8e4
AF = mybir.ActivationFunctionType
DR = mybir.MatmulPerfMode.DoubleRow
MUL = AluOpType.mult
ADD = AluOpType.add

N_EMBED = 768
N_HEAD = 12
HEAD = 64
B, T = 4, 2048
D4 = 4 * N_EMBED          # 3072
HG = N_HEAD // 2          # heads per core = 6
DHG = HG * HEAD           # 384
TOWN = T // 2             # own tokens per core = 1024
GROUPS = [[2 * i, 2 * i + 1] for i in range(4)]
EPS = 1e-5
WS = 64.0                 # fp8 weight pre-scale
IWS = 1.0 / WS

# walrus single-wait-per-instruction limit workaround ------------------------


def _split_ctrl_waits(nc, max_waits=1):
    fn = nc.m.functions[0]
    for bb in fn.blocks:
        insts = list(bb.instructions)
        changed = False
        new_list = []
        for inst in insts:
            si = inst.sync_info
            waits = list(si.on_wait) if (si is not None and si.on_wait) else []
            if len(waits) > max_waits:
                keep = waits[-max_waits:]
                extra = waits[:-max_waits]
                k = 0
                while extra:
                    batch, extra = extra[:max_waits], extra[max_waits:]
                    nop = mybir.InstNoOp(name=f"{inst.name}_wsplit{k}", ins=[], outs=[])
                    nop.engine = inst.engine
                    nop.sync_info = mybir.SyncInfo(on_wait=batch, on_update=[])
                    new_list.append(nop)
                    k += 1
                inst.sync_info = mybir.SyncInfo(
                    on_wait=keep, on_update=list(si.on_update) if si.on_update else []
                )
                changed = True
            new_list.append(inst)
        if changed:
            bb.instructions = new_list


# ---------------------------------------------------------------------------


def _ln_stats4(nc, pools, x_aps):
    """Batched LN stats over 4 [128,768] f32 tiles -> (r, nmr) [128,4] f32."""
    small = pools["small"]
    n = len(x_aps)
    stats = small.tile([128, n, 3, 6], F32, tag="stats")
    for t, x in enumerate(x_aps):
        xv = x.rearrange("p (s d) -> p s d", s=3)
        for s in range(3):
            nc.vector.bn_stats(stats[:, t, s, :], xv[:, s, :])
    mv = small.tile([128, n, 2], F32, tag="mv")
    for t in range(n):
        nc.vector.bn_aggr(mv[:, t, :], stats[:, t, :, :])
    sd = small.tile([128, n], F32, tag="sd")
    nc.scalar.activation(sd, mv[:, :, 1], AF.Sqrt, bias=pools["eps"], scale=1.0)
    r = small.tile([128, n], F32, tag="r")
    nc.vector.reciprocal(r, sd)
    nmr = small.tile([128, n], F32, tag="nmr")
    nc.vector.scalar_tensor_tensor(nmr, mv[:, :, 0], -1.0, r, op0=MUL, op1=MUL)
    return r, nmr


def _ln_normalize(nc, pools, x_ap, r1, nmr1):
    ln_t = pools["lnp"].tile([128, N_EMBED], BF16, tag="ln_t")
    nc.vector.tensor_scalar(ln_t, x_ap, r1, nmr1, op0=MUL, op1=ADD)
    return ln_t


def _ln_transpose(nc, pools, ln_t, dstT, tcol, on_vector):
    """PE-transpose normalized [128,768] into dstT[:, c, tcol:+128] (fp8)."""
    psf = pools["psum_f"]
    ps4 = psf.tile([128, 512], BF16, tag="ps", name="tp4")
    for c in range(4):
        nc.tensor.transpose(
            ps4[:, 128 * c : 128 * (c + 1)], ln_t[:, 128 * c : 128 * (c + 1)],
            pools["ident"],
        )
    ps2 = psf.tile([128, 512], BF16, tag="ps", name="tp2")
    for c in range(2):
        nc.tensor.transpose(
            ps2[:, 128 * c : 128 * (c + 1)], ln_t[:, 512 + 128 * c : 512 + 128 * (c + 1)],
            pools["ident"],
        )
    src4 = ps4.rearrange("p (c x) -> p c x", c=4)
    src2 = ps2[:, 0:256].rearrange("p (c x) -> p c x", c=2)
    if on_vector:
        nc.vector.tensor_copy(dstT[:, 0:4, tcol : tcol + 128], src4)
        nc.vector.tensor_copy(dstT[:, 4:6, tcol : tcol + 128], src2)
    else:
        nc.scalar.copy(dstT[:, 0:4, tcol : tcol + 128], src4)
        nc.scalar.copy(dstT[:, 4:6, tcol : tcol + 128], src2)


def build_nc():
    nc = bass.Bass()

    x_ext = nc.declare_dram_parameter("x", [T, N_EMBED], BF16, isOutput=False)
    xb_ext = nc.declare_dram_parameter("xb", [T, N_EMBED], F32, isOutput=False)
    wq_ext = nc.declare_dram_parameter("wq", [128, 3, 2, DHG], F8, isOutput=False)
    wk_ext = nc.declare_dram_parameter("wk", [128, 3, 2, DHG], F8, isOutput=False)
    wv_ext = nc.declare_dram_parameter("wv", [128, 3, 2, DHG], F8, isOutput=False)
    bqk_ext = nc.declare_dram_parameter("bqk", [128, 6], F32, isOutput=False)
    bv_ext = nc.declare_dram_parameter("bv", [1, DHG], BF16, isOutput=False)
    wap_ext = nc.declare_dram_parameter("wap", [128, 3, 2, N_EMBED], F8, isOutput=False)
    wfc_ext = nc.declare_dram_parameter("wfc", [128, 3, 2, D4], F8, isOutput=False)
    bfc_ext = nc.declare_dram_parameter("bfc", [128, 24], F32, isOutput=False)
    wmp_ext = nc.declare_dram_parameter("wmp", [128, 12, 2, N_EMBED], F8, isOutput=False)
    bmp_ext = nc.declare_dram_parameter("bmp", [1, N_EMBED], BF16, isOutput=False)
    msk_ext = nc.declare_dram_parameter("msk", [128, 4, 512], BF16, isOutput=False)
    out_ext = nc.declare_dram_parameter("out", [TOWN, N_EMBED], F32, isOutput=True)

    y_bounce = nc.dram_tensor("y_bounce", [4, DHG, 512], BF16)
    ag_bounce = nc.dram_tensor("ag_bounce", [4, 2 * DHG, 512], BF16)

    with tile.TileContext(nc) as tc:
        with (
            tc.tile_pool(name="perm", bufs=1) as perm,
            tc.tile_pool(name="small", bufs=3) as small,
            tc.tile_pool(name="psum_s", bufs=2, space="PSUM") as psum_s,
            tc.tile_pool(name="psum_y", bufs=2, space="PSUM") as psum_y,
            tc.tile_pool(name="psum_f", bufs=2, space="PSUM") as psum_f,
            tc.tile_pool(name="lnp", bufs=3) as lnp,
        ):
            # rank within the pair, for dynamic token-quarter addressing
            rank_reg = nc.gpsimd.alloc_register()
            nc.gpsimd.cc_rank_ld(rank_reg, replica_groups=GROUPS)
            rank = nc.gpsimd.snap(rank_reg, donate=True)

            ident = perm.tile([128, 128], BF16, tag="ident")
            make_identity(nc, ident)
            eps_t = perm.tile([128, 1], F32, tag="eps")
            nc.vector.memset(eps_t, EPS)
            ones_row = perm.tile([1, 128], BF16, tag="ones_row")
            nc.vector.memset(ones_row, 1.0)

            # PE warm-up: keep the array busy while the first DMAs land so
            # the HAM clock gate opens before the real matmuls start.
            for wix in range(10):
                pw = psum_s.tile([128, 1024], F32, tag="ps2", name=f"warm{wix}")
                nc.tensor.matmul(pw[:, 0:128], lhsT=ident, rhs=ident, start=True, stop=True)

            msk = perm.tile([128, 4, 512], BF16, tag="msk")
            nc.gpsimd.dma_start(out=msk, in_=msk_ext[:, :, :])
            wq_sb = perm.tile([128, 3, 2, DHG], F8, tag="wq")
            nc.gpsimd.dma_start(out=wq_sb, in_=wq_ext[:, :, :, :])
            wk_sb = perm.tile([128, 3, 2, DHG], F8, tag="wk")
            nc.gpsimd.dma_start(out=wk_sb, in_=wk_ext[:, :, :, :])
            wv_sb = perm.tile([128, 3, 2, DHG], F8, tag="wv")
            nc.gpsimd.dma_start(out=wv_sb, in_=wv_ext[:, :, :, :])
            bqk_sb = perm.tile([128, 6], F32, tag="bqk")
            nc.gpsimd.dma_start(out=bqk_sb, in_=bqk_ext[:, :])
            bv_sb = perm.tile([1, DHG], BF16, tag="bv")
            nc.gpsimd.dma_start(out=bv_sb, in_=bv_ext[:, :])
            wap_sb = perm.tile([128, 3, 2, N_EMBED], F8, tag="wap")
            nc.gpsimd.dma_start(out=wap_sb, in_=wap_ext[:, :, :, :])
            wfc_sb = perm.tile([128, 3, 2, D4], F8, tag="wfc")
            nc.gpsimd.dma_start(out=wfc_sb, in_=wfc_ext[:, :, :, :])
            bfc_sb = perm.tile([128, 24], F32, tag="bfc")
            nc.gpsimd.dma_start(out=bfc_sb, in_=bfc_ext[:, :])
            wmp_sb = perm.tile([128, 12, 2, N_EMBED], F8, tag="wmp")
            nc.gpsimd.dma_start(out=wmp_sb, in_=wmp_ext[:, :, :, :])
            bmp_sb = perm.tile([1, N_EMBED], BF16, tag="bmp")
            nc.gpsimd.dma_start(out=bmp_sb, in_=bmp_ext[:, :])

            pools = {
                "small": small, "psum_f": psum_f, "lnp": lnp,
                "ident": ident, "eps": eps_t,
            }

            with tc.tile_pool(name="qkv", bufs=1) as qkv_pool:
                qT = qkv_pool.tile([128, 3, T], BF16, tag="qT")
                kT = qkv_pool.tile([128, 3, T], BF16, tag="kT")
                v_sb = qkv_pool.tile([128, 16, HG, 2 * HEAD], BF16, tag="v_sb")
                ln1xT = qkv_pool.tile([128, 6, T], F8, tag="ln1xT")

                def qk_group(g):
                    for dst, w_sb, bcol in ((qT, wq_sb, 0), (kT, wk_sb, 3)):
                        for m in range(3):
                            ps = psum_f.tile([128, 512], F32, tag="ps", name=f"qk{g}{m}")
                            for a in range(3):
                                nc.tensor.matmul(
                                    ps,
                                    lhsT=w_sb[:, a, :, 128 * m : 128 * (m + 1)],
                                    rhs=ln1xT[:, 2 * a : 2 * a + 2, 512 * g : 512 * (g + 1)],
                                    start=(a == 0), stop=(a == 2), perf_mode=DR,
                                )
                            nc.scalar.activation(
                                dst[:, m, 512 * g : 512 * (g + 1)], ps, AF.Identity,
                                bias=bqk_sb[:, bcol + m : bcol + m + 1], scale=IWS,
                            )

                def v_group(g):
                    for tl in range(4):
                        t = 4 * g + tl
                        ps = psum_f.tile([128, 512], F32, tag="ps", name=f"v{t}")
                        for a in range(3):
                            nc.tensor.matmul(
                                ps[:, 0:DHG],
                                lhsT=ln1xT[:, 2 * a : 2 * a + 2, 128 * t : 128 * (t + 1)],
                                rhs=wv_sb[:, a, :, :],
                                start=(a == 0), stop=False, perf_mode=DR,
                            )
                        nc.tensor.matmul(
                            ps[:, 0:DHG], lhsT=ones_row, rhs=bv_sb,
                            start=False, stop=True, skip_group_check=True,
                        )
                        nc.vector.tensor_scalar(
                            v_sb[:, t, :, 0:HEAD],
                            ps[:, 0:DHG].rearrange("p (h d) -> p h d", h=HG),
                            IWS, None, op0=MUL,
                        )

                # ===== phase C: attention + interleaved ln1/qkv tail + FFN =====
                with (
                    tc.tile_pool(name="xpool", bufs=6) as xpool,
                    tc.tile_pool(name="attp", bufs=3) as att_pool,
                    tc.tile_pool(name="yTp", bufs=2) as yT_pool,
                    tc.tile_pool(name="yfp", bufs=1) as yf_pool,
                    tc.tile_pool(name="xbp", bufs=4) as xb_pool,
                    tc.tile_pool(name="x1p", bufs=1) as x1_pool,
                    tc.tile_pool(name="ln2p", bufs=1) as ln2_pool,
                    tc.tile_pool(name="hTp", bufs=1) as hT_pool,
                    tc.tile_pool(name="outp", bufs=2) as outp,
                ):
                    xb_view = xb_ext.rearrange(
                        "(ii hh n p) d -> p ii hh n d", ii=2, hh=2, n=4, p=128
                    )
                    ag_v = ag_bounce.rearrange(
                        "(ii hh) (c p) n -> p ii hh c n", ii=2, hh=2, p=128
                    )

                    def a_pieces(g, on_vector):
                        st = {}

                        def dma_stats():
                            xts = []
                            for tl in range(4):
                                t = 4 * g + tl
                                x_t = xpool.tile(
                                    [128, N_EMBED], BF16, tag="x_t", name=f"x{t}"
                                )
                                nc.sync.dma_start(
                                    out=x_t, in_=x_ext[128 * t : 128 * (t + 1), :]
                                )
                                xts.append(x_t)
                            st["xts"] = xts
                            st["r"], st["nmr"] = _ln_stats4(nc, pools, xts)

                        def ln(tl):
                            st[tl] = _ln_normalize(
                                nc, pools, st["xts"][tl], st["r"][:, tl : tl + 1],
                                st["nmr"][:, tl : tl + 1],
                            )

                        def tp(tl):
                            _ln_transpose(
                                nc, pools, st[tl], ln1xT, 128 * (4 * g + tl),
                                on_vector=on_vector,
                            )

                        # normalize (vector) and transpose (PE) in separate
                        # pieces so a drained transpose never stalls the PE
                        # queue waiting on the just-emitted vector op
                        L = [(lambda tl=tl: ln(tl)) for tl in range(4)]
                        P = [(lambda tl=tl: tp(tl)) for tl in range(4)]
                        return [dma_stats, L[0], L[1], P[0], L[2], P[1], L[3], P[2], P[3]]

                    def qk_pieces(g):
                        out = []
                        for dst, w_sb, bcol in ((qT, wq_sb, 0), (kT, wk_sb, 3)):
                            for m in range(3):
                                def p(dst=dst, w_sb=w_sb, bcol=bcol, m=m):
                                    ps = psum_f.tile(
                                        [128, 512], F32, tag="ps", name=f"qk{g}{m}"
                                    )
                                    for a in range(3):
                                        nc.tensor.matmul(
                                            ps,
                                            lhsT=w_sb[:, a, :, 128 * m : 128 * (m + 1)],
                                            rhs=ln1xT[:, 2 * a : 2 * a + 2, 512 * g : 512 * (g + 1)],
                                            start=(a == 0), stop=(a == 2), perf_mode=DR,
                                        )
                                    nc.scalar.activation(
                                        dst[:, m, 512 * g : 512 * (g + 1)], ps,
                                        AF.Identity,
                                        bias=bqk_sb[:, bcol + m : bcol + m + 1],
                                        scale=IWS,
                                    )
                                out.append(p)
                        return out

                    def v_pieces(g):
                        out = []
                        for tl in range(4):
                            def p(tl=tl):
                                t = 4 * g + tl
                                ps = psum_f.tile([128, 512], F32, tag="ps", name=f"v{t}")
                                for a in range(3):
                                    nc.tensor.matmul(
                                        ps[:, 0:DHG],
                                        lhsT=ln1xT[:, 2 * a : 2 * a + 2, 128 * t : 128 * (t + 1)],
                                        rhs=wv_sb[:, a, :, :],
                                        start=(a == 0), stop=False, perf_mode=DR,
                                    )
                                nc.tensor.matmul(
                                    ps[:, 0:DHG], lhsT=ones_row, rhs=bv_sb,
                                    start=False, stop=True, skip_group_check=True,
                                )
                                nc.vector.tensor_scalar(
                                    v_sb[:, t, :, 0:HEAD],
                                    ps[:, 0:DHG].rearrange("p (h d) -> p h d", h=HG),
                                    IWS, None, op0=MUL,
                                )
                            out.append(p)
                        return out

                    # ln1 + qkv for the first quarter up front; everything else
                    # is dripped into the attention instruction stream.
                    for piece in a_pieces(0, False) + a_pieces(1, False):
                        piece()
                    for piece in qk_pieces(0) + v_pieces(0):
                        piece()
                    nc.gpsimd.memset(v_sb[:, :, :, HEAD : 2 * HEAD], 1.0)

                    def ffn_pieces(i):
                        # processes token quarter (2*i + rank): 4 tiles of 128
                        st = {}

                        def dmas():
                            yfT = yf_pool.tile(
                                [128, HG, 512], BF16, tag="yf", name=f"yf{i}"
                            )
                            nc.gpsimd.dma_start(
                                out=yfT, in_=ag_v[:, i, bass.ds(rank, 1), :, :]
                            )
                            xbts = []
                            for tt in range(4):
                                xbt = xb_pool.tile(
                                    [128, N_EMBED], F32, tag="xb", name=f"xb{i}{tt}"
                                )
                                nc.gpsimd.dma_start(
                                    out=xbt, in_=xb_view[:, i, bass.ds(rank, 1), tt, :]
                                )
                                xbts.append(xbt)
                            yf8 = yf_pool.tile(
                                [128, HG, 512], F8, tag="yf8", name=f"yf8{i}"
                            )
                            nc.vector.tensor_copy(yf8, yfT)
                            st["xbts"] = xbts
                            st["yf8"] = yf8

                        def aproj(tt, n0, n1):
                            if "x1" not in st:
                                st["x1"] = x1_pool.tile(
                                    [128, 4, N_EMBED], F32, tag="x1", name=f"x1_{i}"
                                )
                            w = n1 - n0
                            ps = psum_f.tile(
                                [128, 512], F32, tag="ps", name=f"ap{i}{tt}{n0}"
                            )
                            for a in range(3):
                                nc.tensor.matmul(
                                    ps[:, 0:w],
                                    lhsT=st["yf8"][:, 2 * a : 2 * a + 2, 128 * tt : 128 * (tt + 1)],
                                    rhs=wap_sb[:, a, :, n0:n1],
                                    start=(a == 0), stop=(a == 2), perf_mode=DR,
                                )
                            nc.vector.scalar_tensor_tensor(
                                st["x1"][:, tt, n0:n1], ps[:, 0:w], IWS,
                                st["xbts"][tt][:, n0:n1], op0=MUL, op1=ADD,
                            )

                        def ln2_stats():
                            x1 = st["x1"]
                            st["r"], st["nmr"] = _ln_stats4(
                                nc, pools, [x1[:, tt, :] for tt in range(4)]
                            )
                            st["ln2xT"] = ln2_pool.tile(
                                [128, 6, 512], F8, tag="ln2xT", name=f"l2T{i}"
                            )

                        def ln2_apply(tt):
                            ln_t = _ln_normalize(
                                nc, pools, st["x1"][:, tt, :], st["r"][:, tt : tt + 1],
                                st["nmr"][:, tt : tt + 1],
                            )
                            _ln_transpose(
                                nc, pools, ln_t, st["ln2xT"], 128 * tt, on_vector=True,
                            )

                        def fc_part(m0, m1):
                            if "hT" not in st:
                                st["hT"] = hT_pool.tile(
                                    [128, 24, 512], F8, tag="hT", name=f"hT{i}"
                                )
                            for m in range(m0, m1):
                                ps = psum_f.tile(
                                    [128, 512], F32, tag="ps", name=f"fc{i}{m}"
                                )
                                for a in range(3):
                                    nc.tensor.matmul(
                                        ps,
                                        lhsT=wfc_sb[:, a, :, 128 * m : 128 * (m + 1)],
                                        rhs=st["ln2xT"][:, 2 * a : 2 * a + 2, :],
                                        start=(a == 0), stop=(a == 2), perf_mode=DR,
                                    )
                                nc.scalar.activation(
                                    st["hT"][:, m, :], ps, AF.Gelu,
                                    bias=bfc_sb[:, m : m + 1], scale=IWS,
                                )

                        def mproj(tt, n0, n1):
                            if tt not in st:
                                st[tt] = outp.tile(
                                    [128, N_EMBED], F32, tag="o_t", name=f"o{i}{tt}"
                                )
                            o_t = st[tt]
                            w = n1 - n0
                            ps = psum_f.tile(
                                [128, 512], F32, tag="ps", name=f"mp{i}{tt}{n0}"
                            )
                            for a in range(12):
                                nc.tensor.matmul(
                                    ps[:, 0:w],
                                    lhsT=st["hT"][:, 2 * a : 2 * a + 2, 128 * tt : 128 * (tt + 1)],
                                    rhs=wmp_sb[:, a, :, n0:n1],
                                    start=(a == 0), stop=False, perf_mode=DR,
                                )
                            nc.tensor.matmul(
                                ps[:, 0:w], lhsT=ones_row, rhs=bmp_sb[:, n0:n1],
                                start=False, stop=True, skip_group_check=True,
                            )
                            nc.vector.scalar_tensor_tensor(
                                o_t[:, n0:n1], ps[:, 0:w], IWS, st["x1"][:, tt, n0:n1],
                                op0=MUL, op1=ADD,
                            )
                            if n0 > 0:
                                nc.sync.dma_start(
                                    out=out_ext[512 * i + 128 * tt : 512 * i + 128 * (tt + 1), :],
                                    in_=o_t,
                                )

                        early = [dmas]
                        early += [
                            (lambda tt=tt, n0=n0, n1=n1: aproj(tt, n0, n1))
                            for tt in range(4) for n0, n1 in ((0, 512), (512, 768))
                        ]
                        early += [ln2_stats]
                        early += [(lambda tt=tt: ln2_apply(tt)) for tt in range(4)]
                        early += [
                            (lambda m0=m0: fc_part(m0, m0 + 8)) for m0 in (0, 8, 16)
                        ]
                        late = [
                            (lambda tt=tt, n0=n0, n1=n1: mproj(tt, n0, n1))
                            for tt in range(4) for n0, n1 in ((0, 512), (512, 768))
                        ]
                        return early, late

                    import collections as _c
                    fillers = _c.deque()

                    def attn_hp(qc, hp, yTq, drain_from):
                        qoff = 512 * qc
                        nkb = 4 * (qc + 1)
                        ps_y = [
                            psum_y.tile([128, 512], F32, tag="py", name=f"py{qc}{hp}{h2}")
                            for h2 in range(2)
                        ]
                        for kb in range(nkb):
                            ps_s = psum_s.tile([128, 1024], F32, tag="ps2")
                            for h2 in range(2):
                                lo, hi = 64 * h2, 64 * (h2 + 1)
                                nc.tensor.matmul(
                                    ps_s[:, 512 * h2 : 512 * (h2 + 1)],
                                    lhsT=kT[lo:hi, hp, 128 * kb : 128 * (kb + 1)],
                                    rhs=qT[lo:hi, hp, qoff : qoff + 512],
                                    start=True, stop=True,
                                )
                            att = att_pool.tile([128, 1024], BF16, tag="att")
                            nc.scalar.activation(att, ps_s, AF.Exp, scale=0.125)
                            j = kb - 4 * qc
                            if j >= 0:
                                # gpsimd queue is blocked on collectives/gathers
                                # from qc2 on; keep those masks on the DVE
                                eng = nc.gpsimd if qc <= 1 else nc.vector
                                w = 128 * (j + 1)
                                for h2 in range(2):
                                    eng.tensor_mul(
                                        att[:, 512 * h2 : 512 * h2 + w],
                                        att[:, 512 * h2 : 512 * h2 + w],
                                        msk[:, j, 0:w],
                                    )
                            for h2 in range(2):
                                nc.tensor.matmul(
                                    ps_y[h2],
                                    lhsT=v_sb[:, kb, 2 * hp + h2, :],
                                    rhs=att[:, 512 * h2 : 512 * (h2 + 1)],
                                    start=(kb == 0), stop=(kb == nkb - 1),
                                    skip_group_check=True,
                                )
                            if kb >= drain_from and fillers:
                                fillers.popleft()()
                        # copy y/denominator out of PSUM first so the next
                        # hp-group's accumulators can start immediately; the
                        # slow reciprocal then runs off the critical path.
                        yps = att_pool.tile(
                            [128, 1024], F32, tag="yps", bufs=2, name=f"yps{qc}{hp}"
                        )
                        for h2 in range(2):
                            nc.vector.tensor_copy(
                                yps[:, 512 * h2 : 512 * (h2 + 1)], ps_y[h2]
                            )
                        for h2 in range(2):
                            rec = att_pool.tile([HEAD, 512], F32, tag="rec", bufs=1)
                            nc.vector.reciprocal(
                                rec, yps[HEAD : 2 * HEAD, 512 * h2 : 512 * (h2 + 1)]
                            )
                            nc.vector.tensor_mul(
                                yTq[64 * h2 : 64 * (h2 + 1), hp, :],
                                yps[0:HEAD, 512 * h2 : 512 * (h2 + 1)], rec,
                            )

                    def emit_ag(qc, yTq):
                        nc.sync.dma_start(
                            out=y_bounce[qc].rearrange("(c p) n -> p c n", p=128),
                            in_=yTq,
                        )
                        nc.gpsimd.collective_compute(
                            "AllGather",
                            AluOpType.bypass,
                            replica_groups=GROUPS,
                            ins=[y_bounce[qc][:]],
                            outs=[ag_bounce[qc][:]],
                        )

                    # drip the remaining ln1/qkv work into the attention stream
                    fillers.extend(qk_pieces(1))
                    fillers.extend(v_pieces(1))
                    fillers.extend(a_pieces(2, False))
                    fillers.extend(qk_pieces(2))
                    fillers.extend(v_pieces(2))
                    fillers.extend(a_pieces(3, False))
                    fillers.extend(qk_pieces(3))
                    fillers.extend(v_pieces(3))

                    early0 = late0 = None
                    for qc in range(4):
                        yTq = yT_pool.tile([128, 3, 512], BF16, tag="yT", name=f"yT{qc}")
                        for hp in range(3):
                            df = 2 if qc == 2 else 1
                            attn_hp(qc, hp, yTq, df)
                            if hp == 2:
                                emit_ag(qc, yTq)
                            if qc == 2 and hp == 1:
                                # aproj waits on AG1; keep it out of the PE
                                # queue until the gather has had time to land
                                fillers.extend(early0[1:])
                        if qc == 1:
                            early0, late0 = ffn_pieces(0)
                            early0[0]()  # issue block-0 gather/residual DMAs
                        if qc == 2:
                            fillers.extend(late0)  # mproj needs fc's gelus done

                    while fillers:
                        fillers.popleft()()

                    early1, late1 = ffn_pieces(1)
                    for piece in early1 + late1:
                        piece()

    _split_ctrl_waits(nc)
    return nc


_NC_CACHE = None


def _get_nc():
    global _NC_CACHE
    if _NC_CACHE is None:
        _NC_CACHE = build_nc()
    return _NC_CACHE


def _pack_pair(w, f8):
    """[K, M] -> [128, K//256, 2, M] fp8, rows (256a + 128j + p) -> [p, a, j]."""
    K, M = w.shape
    a = np.clip(w * WS, -240.0, 240.0).astype(f8)
    return np.ascontiguousarray(
        a.reshape(K // 256, 2, 128, M).transpose(2, 0, 1, 3)
    )


def _prep_inputs(x, ln1_g, ln1_b, w_attn, b_attn, w_aproj, b_aproj,
                 ln2_g, ln2_b, w_fc, b_fc, w_mproj, b_mproj):
    bf = ml_dtypes.bfloat16
    f32 = np.float32
    f8 = mybir.dt.np(F8)
    x = np.asarray(x, f32)
    ln1_g = np.asarray(ln1_g, f32); ln1_b = np.asarray(ln1_b, f32)
    ln2_g = np.asarray(ln2_g, f32); ln2_b = np.asarray(ln2_b, f32)
    w_attn = np.asarray(w_attn, f32); b_attn = np.asarray(b_attn, f32)
    w_aproj = np.asarray(w_aproj, f32); b_aproj = np.asarray(b_aproj, f32)
    w_fc = np.asarray(w_fc, f32); b_fc = np.asarray(b_fc, f32)
    w_mproj = np.asarray(w_mproj, f32); b_mproj = np.asarray(b_mproj, f32)

    # fold ln1 gain into w_attn rows; ln1 bias into b_attn
    w_attn_f = ln1_g[:, None] * w_attn
    b_attn_f = b_attn + ln1_b @ w_attn
    wq = w_attn_f[:, 0:N_EMBED]; bq = b_attn_f[0:N_EMBED]
    wk = w_attn_f[:, N_EMBED : 2 * N_EMBED]; bk = b_attn_f[N_EMBED : 2 * N_EMBED]
    wv = w_attn_f[:, 2 * N_EMBED :]; bv = b_attn_f[2 * N_EMBED :]

    w_fc_f = ln2_g[:, None] * w_fc
    b_fc_f = b_fc + ln2_b @ w_fc

    # causal diagonal masks in transposed layout: msk[k, j, q] = k + 128j <= q
    kk = np.arange(128)[:, None, None]
    jj = np.arange(4)[None, :, None]
    qq = np.arange(512)[None, None, :]
    msk = ((kk + 128 * jj) <= qq).astype(bf)

    wap_p = _pack_pair(w_aproj, f8)
    wfc_p = _pack_pair(w_fc_f, f8)
    wmp_p = _pack_pair(w_mproj, f8)
    bfc_t = np.ascontiguousarray(b_fc_f.reshape(24, 128).T).astype(f32)
    bmp64 = (WS * b_mproj)[None, :].astype(bf)

    per_rank = []
    for r in range(2):
        hsel = slice(r * DHG, (r + 1) * DHG)
        bqk = np.zeros((128, 6), f32)
        for m in range(3):
            bqk[:, m] = bq[hsel][128 * m : 128 * (m + 1)]
            bqk[:, 3 + m] = bk[hsel][128 * m : 128 * (m + 1)]
        per_rank.append(
            dict(
                wq=_pack_pair(wq[:, hsel], f8),
                wk=_pack_pair(wk[:, hsel], f8),
                wv=_pack_pair(wv[:, hsel], f8),
                bqk=bqk,
                bv=(WS * np.ascontiguousarray(bv[hsel]))[None, :].astype(bf),
                wap=wap_p,
                wfc=wfc_p,
                bfc=bfc_t,
                wmp=wmp_p,
                bmp=bmp64,
                msk=msk,
            )
        )

    in_maps = []
    for c in range(8):
        b_i, r = c // 2, c % 2
        m = dict(per_rank[r])
        m["x"] = np.ascontiguousarray(x[b_i]).astype(bf)
        m["xb"] = np.ascontiguousarray(x[b_i] + b_aproj[None, :])
        in_maps.append(m)
    return in_maps


def kernel(**inputs):
    nc = _get_nc()
    in_maps = _prep_inputs(**inputs)
    res = run_bass_kernel_spmd(nc, in_maps, list(range(8)))
    out = np.empty((B, T, N_EMBED), np.float32)
    for c in range(8):
        b_i, r = c // 2, c % 2
        o = res.results[c]["out"]
        out[b_i, 512 * r : 512 * (r + 1), :] = o[0:512]
        out[b_i, 1024 + 512 * r : 1024 + 512 * (r + 1), :] = o[512:1024]
    return out


# revision 42
# speedup vs baseline: 1.0221x; 1.0221x over previous
"""GPT-2 transformer block on 8 Trainium2 NeuronCores.

Sharding: core c = (batch b = c//2, rank r = c%2).  Pairs (2b, 2b+1) share a
batch: each core computes ln1 + qkv for its 6 of 12 heads over the full
sequence (T=2048), causal attention in transposed layout, an intra-pair
AllGather of per-head outputs per 512-token quarter, then token-parallel
aproj + ln2 + FFN where rank r owns quarters {r, r+2}.  The FFN work for the
first owned quarter is interleaved into the scalar-bound (exp) attention
stream of quarters 2-3 so the PE array never idles.  Dense matmuls (qkv,
aproj, fc, mproj) run in fp8-e4m3 DoubleRow (pairs of adjacent 128-chunks
along the contraction dim); attention scores/AV stay bf16.  Weights are
pre-scaled by 64 before the fp8 cast (avoids the subnormal range) and the
1/64 is folded into the consuming activation/vector op; the attention score
scale 1/sqrt(64) is folded into the exp activation's scale.  LayerNorm
gains/biases and b_aproj are folded into weights / the residual input on the
host.
"""

import numpy as np
import ml_dtypes

import concourse.bass as bass
import concourse.tile as tile
from concourse import mybir
from concourse.alu_op_type import AluOpType
from concourse.masks import make_identity
from concourse.bass_utils import run_bass_kernel_spmd

BF16 = mybir.dt.bfloat16
F32 = mybir.dt.float32
F8 = mybir.dt.float8e4
AF = mybir.ActivationFunctionType
DR = mybir.MatmulPerfMode.DoubleRow
MUL = AluOpType.mult
ADD = AluOpType.add

N_EMBED = 768
N_HEAD = 12
HEAD = 64
B, T = 4, 2048
D4 = 4 * N_EMBED          # 3072
HG = N_HEAD // 2          # heads per core = 6
DHG = HG * HEAD           # 384
TOWN = T // 2             # own tokens per core = 1024
GROUPS = [[2 * i, 2 * i + 1] for i in range(4)]
EPS = 1e-5
WS = 64.0                 # fp8 weight pre-scale
IWS = 1.0 / WS

# walrus single-wait-per-instruction limit workaround ------------------------


def _split_ctrl_waits(nc, max_waits=1):
    fn = nc.m.functions[0]
    for bb in fn.blocks:
        insts = list(bb.instructions)
        changed = False
        new_list = []
        for inst in insts:
            si = inst.sync_info
            waits = list(si.on_wait) if (si is not None and si.on_wait) else []
            if len(waits) > max_waits:
                keep = waits[-max_waits:]
                extra = waits[:-max_waits]
                k = 0
                while extra:
                    batch, extra = extra[:max_waits], extra[max_waits:]
                    nop = mybir.InstNoOp(name=f"{inst.name}_wsplit{k}", ins=[], outs=[])
                    nop.engine = inst.engine
                    nop.sync_info = mybir.SyncInfo(on_wait=batch, on_update=[])
                    new_list.append(nop)
                    k += 1
                inst.sync_info = mybir.SyncInfo(
                    on_wait=keep, on_update=list(si.on_update) if si.on_update else []
                )
                changed = True
            new_list.append(inst)
        if changed:
            bb.instructions = new_list


# ---------------------------------------------------------------------------


def _ln_stats4(nc, pools, x_aps):
    """Batched LN stats over 4 [128,768] f32 tiles -> (r, nmr) [128,4] f32."""
    small = pools["small"]
    n = len(x_aps)
    stats = small.tile([128, n, 3, 6], F32, tag="stats")
    for t, x in enumerate(x_aps):
        xv = x.rearrange("p (s d) -> p s d", s=3)
        for s in range(3):
            nc.vector.bn_stats(stats[:, t, s, :], xv[:, s, :])
    mv = small.tile([128, n, 2], F32, tag="mv")
    for t in range(n):
        nc.vector.bn_aggr(mv[:, t, :], stats[:, t, :, :])
    sd = small.tile([128, n], F32, tag="sd")
    nc.scalar.activation(sd, mv[:, :, 1], AF.Sqrt, bias=pools["eps"], scale=1.0)
    r = small.tile([128, n], F32, tag="r")
    nc.vector.reciprocal(r, sd)
    nmr = small.tile([128, n], F32, tag="nmr")
    nc.vector.scalar_tensor_tensor(nmr, mv[:, :, 0], -1.0, r, op0=MUL, op1=MUL)
    return r, nmr


def _ln_apply_transpose(nc, pools, x_ap, r1, nmr1, dstT, tcol, on_vector):
    """Normalize x [128,768] -> fp8, PE-transpose into dstT[:, c, tcol:+128]."""
    ln_t = pools["lnp"].tile([128, N_EMBED], BF16, tag="ln_t")
    nc.vector.tensor_scalar(ln_t, x_ap, r1, nmr1, op0=MUL, op1=ADD)
    psf = pools["psum_f"]
    ps4 = psf.tile([128, 512], BF16, tag="ps", name="tp4")
    for c in range(4):
        nc.tensor.transpose(
            ps4[:, 128 * c : 128 * (c + 1)], ln_t[:, 128 * c : 128 * (c + 1)],
            pools["ident"],
        )
    ps2 = psf.tile([128, 512], BF16, tag="ps", name="tp2")
    for c in range(2):
        nc.tensor.transpose(
            ps2[:, 128 * c : 128 * (c + 1)], ln_t[:, 512 + 128 * c : 512 + 128 * (c + 1)],
            pools["ident"],
        )
    src4 = ps4.rearrange("p (c x) -> p c x", c=4)
    src2 = ps2[:, 0:256].rearrange("p (c x) -> p c x", c=2)
    if on_vector:
        nc.vector.tensor_copy(dstT[:, 0:4, tcol : tcol + 128], src4)
        nc.vector.tensor_copy(dstT[:, 4:6, tcol : tcol + 128], src2)
    else:
        nc.scalar.copy(dstT[:, 0:4, tcol : tcol + 128], src4)
        nc.scalar.copy(dstT[:, 4:6, tcol : tcol + 128], src2)


def build_nc():
    nc = bass.Bass()

    x_ext = nc.declare_dram_parameter("x", [T, N_EMBED], BF16, isOutput=False)
    xb_ext = nc.declare_dram_parameter("xb", [T, N_EMBED], F32, isOutput=False)
    wq_ext = nc.declare_dram_parameter("wq", [128, 3, 2, DHG], F8, isOutput=False)
    wk_ext = nc.declare_dram_parameter("wk", [128, 3, 2, DHG], F8, isOutput=False)
    wv_ext = nc.declare_dram_parameter("wv", [128, 3, 2, DHG], F8, isOutput=False)
    bqk_ext = nc.declare_dram_parameter("bqk", [128, 6], F32, isOutput=False)
    bv_ext = nc.declare_dram_parameter("bv", [1, DHG], BF16, isOutput=False)
    wap_ext = nc.declare_dram_parameter("wap", [128, 3, 2, N_EMBED], F8, isOutput=False)
    wfc_ext = nc.declare_dram_parameter("wfc", [128, 3, 2, D4], F8, isOutput=False)
    bfc_ext = nc.declare_dram_parameter("bfc", [128, 24], F32, isOutput=False)
    wmp_ext = nc.declare_dram_parameter("wmp", [128, 12, 2, N_EMBED], F8, isOutput=False)
    bmp_ext = nc.declare_dram_parameter("bmp", [1, N_EMBED], BF16, isOutput=False)
    msk_ext = nc.declare_dram_parameter("msk", [128, 4, 512], BF16, isOutput=False)
    out_ext = nc.declare_dram_parameter("out", [TOWN, N_EMBED], F32, isOutput=True)

    y_bounce = nc.dram_tensor("y_bounce", [4, DHG, 512], BF16)
    ag_bounce = nc.dram_tensor("ag_bounce", [4, 2 * DHG, 512], BF16)

    with tile.TileContext(nc) as tc:
        with (
            tc.tile_pool(name="perm", bufs=1) as perm,
            tc.tile_pool(name="small", bufs=3) as small,
            tc.tile_pool(name="psum_s", bufs=2, space="PSUM") as psum_s,
            tc.tile_pool(name="psum_y", bufs=2, space="PSUM") as psum_y,
            tc.tile_pool(name="psum_f", bufs=2, space="PSUM") as psum_f,
            tc.tile_pool(name="lnp", bufs=2) as lnp,
        ):
            # rank within the pair, for dynamic token-quarter addressing
            rank_reg = nc.gpsimd.alloc_register()
            nc.gpsimd.cc_rank_ld(rank_reg, replica_groups=GROUPS)
            rank = nc.gpsimd.snap(rank_reg, donate=True)

            ident = perm.tile([128, 128], BF16, tag="ident")
            make_identity(nc, ident)
            eps_t = perm.tile([128, 1], F32, tag="eps")
            nc.vector.memset(eps_t, EPS)
            ones_row = perm.tile([1, 128], BF16, tag="ones_row")
            nc.vector.memset(ones_row, 1.0)

            # PE warm-up: keep the array busy while the first DMAs land so
            # the HAM clock gate opens before the real matmuls start.
            for wix in range(10):
                pw = psum_s.tile([128, 1024], F32, tag="ps2", name=f"warm{wix}")
                nc.tensor.matmul(pw[:, 0:128], lhsT=ident, rhs=ident, start=True, stop=True)

            msk = perm.tile([128, 4, 512], BF16, tag="msk")
            nc.gpsimd.dma_start(out=msk, in_=msk_ext[:, :, :])
            wq_sb = perm.tile([128, 3, 2, DHG], F8, tag="wq")
            nc.gpsimd.dma_start(out=wq_sb, in_=wq_ext[:, :, :, :])
            wk_sb = perm.tile([128, 3, 2, DHG], F8, tag="wk")
            nc.gpsimd.dma_start(out=wk_sb, in_=wk_ext[:, :, :, :])
            wv_sb = perm.tile([128, 3, 2, DHG], F8, tag="wv")
            nc.gpsimd.dma_start(out=wv_sb, in_=wv_ext[:, :, :, :])
            bqk_sb = perm.tile([128, 6], F32, tag="bqk")
            nc.gpsimd.dma_start(out=bqk_sb, in_=bqk_ext[:, :])
            bv_sb = perm.tile([1, DHG], BF16, tag="bv")
            nc.gpsimd.dma_start(out=bv_sb, in_=bv_ext[:, :])
            wap_sb = perm.tile([128, 3, 2, N_EMBED], F8, tag="wap")
            nc.gpsimd.dma_start(out=wap_sb, in_=wap_ext[:, :, :, :])
            wfc_sb = perm.tile([128, 3, 2, D4], F8, tag="wfc")
            nc.gpsimd.dma_start(out=wfc_sb, in_=wfc_ext[:, :, :, :])
            bfc_sb = perm.tile([128, 24], F32, tag="bfc")
            nc.gpsimd.dma_start(out=bfc_sb, in_=bfc_ext[:, :])
            wmp_sb = perm.tile([128, 12, 2, N_EMBED], F8, tag="wmp")
            nc.gpsimd.dma_start(out=wmp_sb, in_=wmp_ext[:, :, :, :])
            bmp_sb = perm.tile([1, N_EMBED], BF16, tag="bmp")
            nc.gpsimd.dma_start(out=bmp_sb, in_=bmp_ext[:, :])

            pools = {
                "small": small, "psum_f": psum_f, "lnp": lnp,
                "ident": ident, "eps": eps_t,
            }

            with tc.tile_pool(name="qkv", bufs=1) as qkv_pool:
                qT = qkv_pool.tile([128, 3, T], BF16, tag="qT")
                kT = qkv_pool.tile([128, 3, T], BF16, tag="kT")
                v_sb = qkv_pool.tile([128, 16, HG, 2 * HEAD], BF16, tag="v_sb")
                ln1xT = qkv_pool.tile([128, 6, T], F8, tag="ln1xT")

                def qk_group(g):
                    for dst, w_sb, bcol in ((qT, wq_sb, 0), (kT, wk_sb, 3)):
                        for m in range(3):
                            ps = psum_f.tile([128, 512], F32, tag="ps", name=f"qk{g}{m}")
                            for a in range(3):
                                nc.tensor.matmul(
                                    ps,
                                    lhsT=w_sb[:, a, :, 128 * m : 128 * (m + 1)],
                                    rhs=ln1xT[:, 2 * a : 2 * a + 2, 512 * g : 512 * (g + 1)],
                                    start=(a == 0), stop=(a == 2), perf_mode=DR,
                                )
                            nc.scalar.activation(
                                dst[:, m, 512 * g : 512 * (g + 1)], ps, AF.Identity,
                                bias=bqk_sb[:, bcol + m : bcol + m + 1], scale=IWS,
                            )

                def v_group(g):
                    for tl in range(4):
                        t = 4 * g + tl
                        ps = psum_f.tile([128, 512], F32, tag="ps", name=f"v{t}")
                        for a in range(3):
                            nc.tensor.matmul(
                                ps[:, 0:DHG],
                                lhsT=ln1xT[:, 2 * a : 2 * a + 2, 128 * t : 128 * (t + 1)],
                                rhs=wv_sb[:, a, :, :],
                                start=(a == 0), stop=False, perf_mode=DR,
                            )
                        nc.tensor.matmul(
                            ps[:, 0:DHG], lhsT=ones_row, rhs=bv_sb,
                            start=False, stop=True, skip_group_check=True,
                        )
                        nc.vector.tensor_scalar(
                            v_sb[:, t, :, 0:HEAD],
                            ps[:, 0:DHG].rearrange("p (h d) -> p h d", h=HG),
                            IWS, None, op0=MUL,
                        )

                # ===== phase C: attention + interleaved ln1/qkv tail + FFN =====
                with (
                    tc.tile_pool(name="xpool", bufs=6) as xpool,
                    tc.tile_pool(name="attp", bufs=3) as att_pool,
                    tc.tile_pool(name="yTp", bufs=2) as yT_pool,
                    tc.tile_pool(name="yfp", bufs=1) as yf_pool,
                    tc.tile_pool(name="xbp", bufs=4) as xb_pool,
                    tc.tile_pool(name="x1p", bufs=1) as x1_pool,
                    tc.tile_pool(name="ln2p", bufs=1) as ln2_pool,
                    tc.tile_pool(name="hTp", bufs=1) as hT_pool,
                    tc.tile_pool(name="outp", bufs=2) as outp,
                ):
                    xb_view = xb_ext.rearrange(
                        "(ii hh n p) d -> p ii hh n d", ii=2, hh=2, n=4, p=128
                    )
                    ag_v = ag_bounce.rearrange(
                        "(ii hh) (c p) n -> p ii hh c n", ii=2, hh=2, p=128
                    )

                    def a_pieces(g, on_vector):
                        st = {}

                        def dma_stats():
                            xts = []
                            for tl in range(4):
                                t = 4 * g + tl
                                x_t = xpool.tile(
                                    [128, N_EMBED], BF16, tag="x_t", name=f"x{t}"
                                )
                                nc.sync.dma_start(
                                    out=x_t, in_=x_ext[128 * t : 128 * (t + 1), :]
                                )
                                xts.append(x_t)
                            st["xts"] = xts
                            st["r"], st["nmr"] = _ln_stats4(nc, pools, xts)

                        def ap(tl):
                            _ln_apply_transpose(
                                nc, pools, st["xts"][tl], st["r"][:, tl : tl + 1],
                                st["nmr"][:, tl : tl + 1], ln1xT, 128 * (4 * g + tl),
                                on_vector=on_vector,
                            )

                        return [dma_stats] + [
                            (lambda tl=tl: ap(tl)) for tl in range(4)
                        ]

                    def qk_pieces(g):
                        out = []
                        for dst, w_sb, bcol in ((qT, wq_sb, 0), (kT, wk_sb, 3)):
                            for m in range(3):
                                def p(dst=dst, w_sb=w_sb, bcol=bcol, m=m):
                                    ps = psum_f.tile(
                                        [128, 512], F32, tag="ps", name=f"qk{g}{m}"
                                    )
                                    for a in range(3):
                                        nc.tensor.matmul(
                                            ps,
                                            lhsT=w_sb[:, a, :, 128 * m : 128 * (m + 1)],
                                            rhs=ln1xT[:, 2 * a : 2 * a + 2, 512 * g : 512 * (g + 1)],
                                            start=(a == 0), stop=(a == 2), perf_mode=DR,
                                        )
                                    nc.scalar.activation(
                                        dst[:, m, 512 * g : 512 * (g + 1)], ps,
                                        AF.Identity,
                                        bias=bqk_sb[:, bcol + m : bcol + m + 1],
                                        scale=IWS,
                                    )
                                out.append(p)
                        return out

                    def v_pieces(g):
                        out = []
                        for tl in range(4):
                            def p(tl=tl):
                                t = 4 * g + tl
                                ps = psum_f.tile([128, 512], F32, tag="ps", name=f"v{t}")
                                for a in range(3):
                                    nc.tensor.matmul(
                                        ps[:, 0:DHG],
                                        lhsT=ln1xT[:, 2 * a : 2 * a + 2, 128 * t : 128 * (t + 1)],
                                        rhs=wv_sb[:, a, :, :],
                                        start=(a == 0), stop=False, perf_mode=DR,
                                    )
                                nc.tensor.matmul(
                                    ps[:, 0:DHG], lhsT=ones_row, rhs=bv_sb,
                                    start=False, stop=True, skip_group_check=True,
                                )
                                nc.vector.tensor_scalar(
                                    v_sb[:, t, :, 0:HEAD],
                                    ps[:, 0:DHG].rearrange("p (h d) -> p h d", h=HG),
                                    IWS, None, op0=MUL,
                                )
                            out.append(p)
                        return out

                    # ln1 + qkv for the first quarter up front; everything else
                    # is dripped into the attention instruction stream.
                    for piece in a_pieces(0, False) + a_pieces(1, False):
                        piece()
                    for piece in qk_pieces(0) + v_pieces(0):
                        piece()
                    nc.vector.memset(v_sb[:, :, :, HEAD : 2 * HEAD], 1.0)

                    def ffn_pieces(i):
                        # processes token quarter (2*i + rank): 4 tiles of 128
                        st = {}

                        def dmas():
                            yfT = yf_pool.tile(
                                [128, HG, 512], BF16, tag="yf", name=f"yf{i}"
                            )
                            nc.gpsimd.dma_start(
                                out=yfT, in_=ag_v[:, i, bass.ds(rank, 1), :, :]
                            )
                            xbts = []
                            for tt in range(4):
                                xbt = xb_pool.tile(
                                    [128, N_EMBED], F32, tag="xb", name=f"xb{i}{tt}"
                                )
                                nc.gpsimd.dma_start(
                                    out=xbt, in_=xb_view[:, i, bass.ds(rank, 1), tt, :]
                                )
                                xbts.append(xbt)
                            yf8 = yf_pool.tile(
                                [128, HG, 512], F8, tag="yf8", name=f"yf8{i}"
                            )
                            nc.vector.tensor_copy(yf8, yfT)
                            st["xbts"] = xbts
                            st["yf8"] = yf8

                        def aproj(tt, n0, n1):
                            if "x1" not in st:
                                st["x1"] = x1_pool.tile(
                                    [128, 4, N_EMBED], F32, tag="x1", name=f"x1_{i}"
                                )
                            w = n1 - n0
                            ps = psum_f.tile(
                                [128, 512], F32, tag="ps", name=f"ap{i}{tt}{n0}"
                            )
                            for a in range(3):
                                nc.tensor.matmul(
                                    ps[:, 0:w],
                                    lhsT=st["yf8"][:, 2 * a : 2 * a + 2, 128 * tt : 128 * (tt + 1)],
                                    rhs=wap_sb[:, a, :, n0:n1],
                                    start=(a == 0), stop=(a == 2), perf_mode=DR,
                                )
                            nc.vector.scalar_tensor_tensor(
                                st["x1"][:, tt, n0:n1], ps[:, 0:w], IWS,
                                st["xbts"][tt][:, n0:n1], op0=MUL, op1=ADD,
                            )

                        def ln2_stats():
                            x1 = st["x1"]
                            st["r"], st["nmr"] = _ln_stats4(
                                nc, pools, [x1[:, tt, :] for tt in range(4)]
                            )
                            st["ln2xT"] = ln2_pool.tile(
                                [128, 6, 512], F8, tag="ln2xT", name=f"l2T{i}"
                            )

                        def ln2_apply(tt):
                            _ln_apply_transpose(
                                nc, pools, st["x1"][:, tt, :], st["r"][:, tt : tt + 1],
                                st["nmr"][:, tt : tt + 1], st["ln2xT"], 128 * tt,
                                on_vector=True,
                            )

                        def fc_part(m0, m1):
                            if "hT" not in st:
                                st["hT"] = hT_pool.tile(
                                    [128, 24, 512], F8, tag="hT", name=f"hT{i}"
                                )
                            for m in range(m0, m1):
                                ps = psum_f.tile(
                                    [128, 512], F32, tag="ps", name=f"fc{i}{m}"
                                )
                                for a in range(3):
                                    nc.tensor.matmul(
                                        ps,
                                        lhsT=wfc_sb[:, a, :, 128 * m : 128 * (m + 1)],
                                        rhs=st["ln2xT"][:, 2 * a : 2 * a + 2, :],
                                        start=(a == 0), stop=(a == 2), perf_mode=DR,
                                    )
                                nc.scalar.activation(
                                    st["hT"][:, m, :], ps, AF.Gelu,
                                    bias=bfc_sb[:, m : m + 1], scale=IWS,
                                )

                        def mproj(tt, n0, n1):
                            if tt not in st:
                                st[tt] = outp.tile(
                                    [128, N_EMBED], F32, tag="o_t", name=f"o{i}{tt}"
                                )
                            o_t = st[tt]
                            w = n1 - n0
                            ps = psum_f.tile(
                                [128, 512], F32, tag="ps", name=f"mp{i}{tt}{n0}"
                            )
                            for a in range(12):
                                nc.tensor.matmul(
                                    ps[:, 0:w],
                                    lhsT=st["hT"][:, 2 * a : 2 * a + 2, 128 * tt : 128 * (tt + 1)],
                                    rhs=wmp_sb[:, a, :, n0:n1],
                                    start=(a == 0), stop=False, perf_mode=DR,
                                )
                            nc.tensor.matmul(
                                ps[:, 0:w], lhsT=ones_row, rhs=bmp_sb[:, n0:n1],
                                start=False, stop=True, skip_group_check=True,
                            )
                            nc.vector.scalar_tensor_tensor(
                                o_t[:, n0:n1], ps[:, 0:w], IWS, st["x1"][:, tt, n0:n1],
                                op0=MUL, op1=ADD,
                            )
                            if n0 > 0:
                                nc.sync.dma_start(
                                    out=out_ext[512 * i + 128 * tt : 512 * i + 128 * (tt + 1), :],
                                    in_=o_t,
                                )

                        early = [dmas]
                        early += [
                            (lambda tt=tt, n0=n0, n1=n1: aproj(tt, n0, n1))
                            for tt in range(4) for n0, n1 in ((0, 512), (512, 768))
                        ]
                        early += [ln2_stats]
                        early += [(lambda tt=tt: ln2_apply(tt)) for tt in range(4)]
                        early += [
                            (lambda m0=m0: fc_part(m0, m0 + 8)) for m0 in (0, 8, 16)
                        ]
                        late = [
                            (lambda tt=tt, n0=n0, n1=n1: mproj(tt, n0, n1))
                            for tt in range(4) for n0, n1 in ((0, 512), (512, 768))
                        ]
                        return early, late

                    import collections as _c
                    fillers = _c.deque()

                    def attn_hp(qc, hp, yTq, drain_from):
                        qoff = 512 * qc
                        nkb = 4 * (qc + 1)
                        ps_y = [
                            psum_y.tile([128, 512], F32, tag="py", name=f"py{qc}{hp}{h2}")
                            for h2 in range(2)
                        ]
                        for kb in range(nkb):
                            ps_s = psum_s.tile([128, 1024], F32, tag="ps2")
                            for h2 in range(2):
                                lo, hi = 64 * h2, 64 * (h2 + 1)
                                nc.tensor.matmul(
                                    ps_s[:, 512 * h2 : 512 * (h2 + 1)],
                                    lhsT=kT[lo:hi, hp, 128 * kb : 128 * (kb + 1)],
                                    rhs=qT[lo:hi, hp, qoff : qoff + 512],
                                    start=True, stop=True,
                                )
                            att = att_pool.tile([128, 1024], BF16, tag="att")
                            nc.scalar.activation(att, ps_s, AF.Exp, scale=0.125)
                            j = kb - 4 * qc
                            if j >= 0:
                                # gpsimd queue is blocked on collectives/gathers
                                # from qc2 on; keep those masks on the DVE
                                eng = nc.gpsimd if qc <= 1 else nc.vector
                                w = 128 * (j + 1)
                                for h2 in range(2):
                                    eng.tensor_mul(
                                        att[:, 512 * h2 : 512 * h2 + w],
                                        att[:, 512 * h2 : 512 * h2 + w],
                                        msk[:, j, 0:w],
                                    )
                            for h2 in range(2):
                                nc.tensor.matmul(
                                    ps_y[h2],
                                    lhsT=v_sb[:, kb, 2 * hp + h2, :],
                                    rhs=att[:, 512 * h2 : 512 * (h2 + 1)],
                                    start=(kb == 0), stop=(kb == nkb - 1),
                                    skip_group_check=True,
                                )
                            if kb >= drain_from and fillers:
                                fillers.popleft()()
                        # copy y/denominator out of PSUM first so the next
                        # hp-group's accumulators can start immediately; the
                        # slow reciprocal then runs off the critical path.
                        yps = att_pool.tile(
                            [128, 1024], F32, tag="yps", bufs=2, name=f"yps{qc}{hp}"
                        )
                        for h2 in range(2):
                            nc.vector.tensor_copy(
                                yps[:, 512 * h2 : 512 * (h2 + 1)], ps_y[h2]
                            )
                        for h2 in range(2):
                            rec = att_pool.tile([HEAD, 512], F32, tag="rec", bufs=1)
                            nc.vector.reciprocal(
                                rec, yps[HEAD : 2 * HEAD, 512 * h2 : 512 * (h2 + 1)]
                            )
                            nc.vector.tensor_mul(
                                yTq[64 * h2 : 64 * (h2 + 1), hp, :],
                                yps[0:HEAD, 512 * h2 : 512 * (h2 + 1)], rec,
                            )

                    def emit_ag(qc, yTq):
                        nc.sync.dma_start(
                            out=y_bounce[qc].rearrange("(c p) n -> p c n", p=128),
                            in_=yTq,
                        )
                        nc.gpsimd.collective_compute(
                            "AllGather",
                            AluOpType.bypass,
                            replica_groups=GROUPS,
                            ins=[y_bounce[qc][:]],
                            outs=[ag_bounce[qc][:]],
                        )

                    # drip the remaining ln1/qkv work into the attention stream
                    fillers.extend(qk_pieces(1))
                    fillers.extend(v_pieces(1))
                    fillers.extend(a_pieces(2, False))
                    fillers.extend(qk_pieces(2))
                    fillers.extend(v_pieces(2))
                    fillers.extend(a_pieces(3, False))
                    fillers.extend(qk_pieces(3))
                    fillers.extend(v_pieces(3))

                    early0 = late0 = None
                    for qc in range(4):
                        yTq = yT_pool.tile([128, 3, 512], BF16, tag="yT", name=f"yT{qc}")
                        for hp in range(3):
                            df = 2 if qc == 2 else 1
                            attn_hp(qc, hp, yTq, df)
                            if hp == 2:
                                emit_ag(qc, yTq)
                            if qc == 2 and hp == 1:
                                # aproj waits on AG1; keep it out of the PE
                                # queue until the gather has had time to land
                                fillers.extend(early0[1:])
                        if qc == 1:
                            early0, late0 = ffn_pieces(0)
                            early0[0]()  # issue block-0 gather/residual DMAs
                        if qc == 2:
                            fillers.extend(late0)  # mproj needs fc's gelus done

                    while fillers:
                        fillers.popleft()()

                    early1, late1 = ffn_pieces(1)
                    for piece in early1 + late1:
                        piece()

    _split_ctrl_waits(nc)
    return nc


_NC_CACHE = None


def _get_nc():
    global _NC_CACHE
    if _NC_CACHE is None:
        _NC_CACHE = build_nc()
    return _NC_CACHE


def _pack_pair(w, f8):
    """[K, M] -> [128, K//256, 2, M] fp8, rows (256a + 128j + p) -> [p, a, j]."""
    K, M = w.shape
    a = np.clip(w * WS, -240.0, 240.0).astype(f8)
    return np.ascontiguousarray(
        a.reshape(K // 256, 2, 128, M).transpose(2, 0, 1, 3)
    )


def _prep_inputs(x, ln1_g, ln1_b, w_attn, b_attn, w_aproj, b_aproj,
                 ln2_g, ln2_b, w_fc, b_fc, w_mproj, b_mproj):
    bf = ml_dtypes.bfloat16
    f32 = np.float32
    f8 = mybir.dt.np(F8)
    x = np.asarray(x, f32)
    ln1_g = np.asarray(ln1_g, f32); ln1_b = np.asarray(ln1_b, f32)
    ln2_g = np.asarray(ln2_g, f32); ln2_b = np.asarray(ln2_b, f32)
    w_attn = np.asarray(w_attn, f32); b_attn = np.asarray(b_attn, f32)
    w_aproj = np.asarray(w_aproj, f32); b_aproj = np.asarray(b_aproj, f32)
    w_fc = np.asarray(w_fc, f32); b_fc = np.asarray(b_fc, f32)
    w_mproj = np.asarray(w_mproj, f32); b_mproj = np.asarray(b_mproj, f32)

    # fold ln1 gain into w_attn rows; ln1 bias into b_attn
    w_attn_f = ln1_g[:, None] * w_attn
    b_attn_f = b_attn + ln1_b @ w_attn
    wq = w_attn_f[:, 0:N_EMBED]; bq = b_attn_f[0:N_EMBED]
    wk = w_attn_f[:, N_EMBED : 2 * N_EMBED]; bk = b_attn_f[N_EMBED : 2 * N_EMBED]
    wv = w_attn_f[:, 2 * N_EMBED :]; bv = b_attn_f[2 * N_EMBED :]

    w_fc_f = ln2_g[:, None] * w_fc
    b_fc_f = b_fc + ln2_b @ w_fc

    # causal diagonal masks in transposed layout: msk[k, j, q] = k + 128j <= q
    kk = np.arange(128)[:, None, None]
    jj = np.arange(4)[None, :, None]
    qq = np.arange(512)[None, None, :]
    msk = ((kk + 128 * jj) <= qq).astype(bf)

    wap_p = _pack_pair(w_aproj, f8)
    wfc_p = _pack_pair(w_fc_f, f8)
    wmp_p = _pack_pair(w_mproj, f8)
    bfc_t = np.ascontiguousarray(b_fc_f.reshape(24, 128).T).astype(f32)
    bmp64 = (WS * b_mproj)[None, :].astype(bf)

    per_rank = []
    for r in range(2):
        hsel = slice(r * DHG, (r + 1) * DHG)
        bqk = np.zeros((128, 6), f32)
        for m in range(3):
            bqk[:, m] = bq[hsel][128 * m : 128 * (m + 1)]
            bqk[:, 3 + m] = bk[hsel][128 * m : 128 * (m + 1)]
        per_rank.append(
            dict(
                wq=_pack_pair(wq[:, hsel], f8),
                wk=_pack_pair(wk[:, hsel], f8),
                wv=_pack_pair(wv[:, hsel], f8),
                bqk=bqk,
                bv=(WS * np.ascontiguousarray(bv[hsel]))[None, :].astype(bf),
                wap=wap_p,
                wfc=wfc_p,
                bfc=bfc_t,
                wmp=wmp_p,
                bmp=bmp64,
                msk=msk,
            )
        )

    in_maps = []
    for c in range(8):
        b_i, r = c // 2, c % 2
        m = dict(per_rank[r])
        m["x"] = np.ascontiguousarray(x[b_i]).astype(bf)
        m["xb"] = np.ascontiguousarray(x[b_i] + b_aproj[None, :])
        in_maps.append(m)
    return in_maps


def kernel(**inputs):
    nc = _get_nc()
    in_maps = _prep_inputs(**inputs)
    res = run_bass_kernel_spmd(nc, in_maps, list(range(8)))
    out = np.empty((B, T, N_EMBED), np.float32)
    for c in range(8):
        b_i, r = c // 2, c % 2
        o = res.results[c]["out"]
        out[b_i, 512 * r : 512 * (r + 1), :] = o[0:512]
        out[b_i, 1024 + 512 * r : 1024 + 512 * (r + 1), :] = o[512:1024]
    return out
